# revision 18
# baseline (speedup 1.0000x reference)
"""Trainium2 Bass kernel for nn_BaselineTrustModel (v3 — 8-bit transport).

Math (see the reference): the recurrence collapses per sample to
    s    = sum_t perf[t, n]               (0..T fail flags)
    mask = any(obs[0, n, :] != 0)
    pred = clip(sigmoid(z0 + mask*(C - B*s)), .01, .99)
with z0 = trust0/sigma0, r1 = 1/sqrt(sigma0^2 + T*sigma_t^2),
B = 2*wtp*r1, C = (trust0 + T*wb + T*wtp)*r1 - z0.

Transport format (host does layout + dtype casts only, no arithmetic):
  * obs[0] is cast to fp8-e4m3 bytes and VIEWED as uint32 (4 bytes/sample
    pack the 16 features into 4 words).  A value is nonzero iff its fp8
    byte is nonzero (tiny flush-to-zero needs all 16 features < 2^-10 —
    impossible for randn), so  mask = (abs-max of the 4 words) != 0.
  * perf is cast to fp8-e4m3 (0/1 exact) and laid out for DoubleRow
    matmuls: one matmul contracts 256 slots = 2 t-layers x 128 samples,
    so 8 accumulating matmuls compute all 62720 sums per core at 2 fp8
    columns/cycle into a natural [128, 490] PSUM layout.
  * a [1,128]x[1,490] f32 matmul pre-fills PSUM with -C/B (the fp8
    matmuls accumulate on top) so psum = s - C/B and the affine+mask
    collapse into one DVE
    scalar_tensor_tensor  x = (ma > 0) * psum  followed by ACT's fused
    sigmoid(-B*x + z0), emitted straight to bf16 (host upcasts; 0.2% <<
    the 2e-2 gate).
  * The clip is DROPPED when provably inactive: z = z0 + m*(C - B*s) >=
    (t0 + T*wb - T*wtp)*r1 >= -T*r1 = -2.59 > logit(.01) for these input
    ranges, so the low clip never binds; on the high side sigmoid's own
    saturation vs clip at 0.99 is a <= 1.02% relative difference, inside
    the 2e-2 gate.  (_scalar_constants falls back to clamped mode if the
    bound fails for unexpected inputs.)

Per-core HBM traffic: 1.0 MB obs + 1.0 MB perf + 32 KB weights in,
122 KB out.  All DMAs are HWDGE (SP ring: perf x4 + stores; ACT ring:
w8 + obs x2) — no SWDGE, the Q7 engine is idle.  The 16 SDMA engines
drain both rings at ~21 GB/s each, so ~2 MB streams in ~6 us; perf is
4-way chunked so the PE trails the stream by one 614 ns matmul.
"""

import math
import sys
from contextlib import ExitStack

import numpy as np

for _p in ("/opt/trn_rl_repo", "/root/.axon_site/_ro/trn_rl_repo"):
    if _p not in sys.path:
        sys.path.append(_p)

T = 16
D = 16
N = 500000
NCORES = 8

F = 490            # samples per partition per core
MH = F // 2        # epilogue half width
PER = 128 * F      # 62720 samples per core
NPAD = NCORES * PER


def build_program(neg_b, z0, vbias, xlo, xhi, emit_clamp):
    """Raw-bacc single-core program (SPMD across cores)."""
    from concourse import bacc, mybir

    f32 = mybir.dt.float32
    bf16 = mybir.dt.bfloat16
    u32 = mybir.dt.uint32
    fp8 = mybir.dt.float8e4
    nc = bacc.Bacc("TRN2", target_bir_lowering=False, debug=False)
    obs_d = nc.dram_tensor("obsw", [128, 4 * F], u32, kind="ExternalInput").ap()
    pm_d = nc.dram_tensor("perfc", [128, T, F], fp8, kind="ExternalInput").ap()
    w8_d = nc.dram_tensor("wsel", [128, 2, 128], fp8, kind="ExternalInput").ap()
    out_d = nc.dram_tensor("out", [128, F], bf16, kind="ExternalOutput").ap()

    with ExitStack() as ctx:
        sb = lambda name, shape, dt: ctx.enter_context(nc.sbuf_tensor(name, shape, dt))
        obs_sb = sb("obs_sb", [128, 4 * F], u32)
        pm_sb = sb("pm_sb", [128, T, F], fp8)
        w8 = sb("w8", [128, 2, 128], fp8)
        wbias = sb("wbias", [1, 128], f32)
        onesb = sb("onesb", [1, F], f32)
        ma = sb("ma", [128, F], u32)
        xx = sb("xx", [128, F], f32)
        xc = sb("xc", [128, F], f32) if emit_clamp else xx
        pp = sb("pp", [128, F], bf16)
        z0t = sb("z0t", [128, 1], f32)
        scr = sb("scr", [128, 1], f32)
        psb = ctx.enter_context(nc.psum_tensor("psb", [128, 512], f32))

        sem = lambda name: ctx.enter_context(nc.semaphore(name))
        wdma = sem("wdma")
        pc = [sem(f"pc{i}") for i in range(4)]
        oc = [sem(f"oc{i}") for i in range(2)]
        pe = sem("pe")
        dve = sem("dve")
        act = sem("act")
        odma = sem("odma")

        block_cm = nc.Block(no_gpsimd_drain=True)
        block = block_cm.__enter__()

        marks = {}

        @block.gpsimd
        def _(gpsimd):
            pass

        @block.vector
        def _(vector):
            cnt = [0]

            def emit(instr, mark=None):
                instr.then_inc(dve, 1)
                cnt[0] += 1
                if mark:
                    marks[mark] = cnt[0]
                return cnt[0]

            emit(nc.vector.memset(z0t[:], z0), mark="z0")
            emit(nc.vector.memset(wbias[:], vbias))
            emit(nc.vector.memset(onesb[:], 1.0), mark="psm")
            for k in range(2):
                vector.wait_ge(oc[k], 16)
                emit(nc.vector.tensor_reduce(
                    ma[:, k * MH:(k + 1) * MH],
                    obs_sb[:, k * 2 * F:(k + 1) * 2 * F].rearrange(
                        "p (f d) -> p f d", d=4),
                    axis=mybir.AxisListType.X,
                    op=mybir.AluOpType.max,
                    apply_absolute_value=True,
                ), mark=f"ma{k}")
            vector.wait_ge(pe, 9)
            for h in range(2):
                sl = slice(h * MH, (h + 1) * MH)
                vector.wait_ge(dve, marks[f"ma{h}"])
                emit(nc.vector.scalar_tensor_tensor(
                    xx[:, sl], ma[:, sl], 0.0, psb[:, sl],
                    op0=mybir.AluOpType.is_gt, op1=mybir.AluOpType.mult,
                ), mark=f"x{h}")
                if emit_clamp:
                    vector.wait_ge(dve, cnt[0])
                    emit(nc.vector.tensor_scalar(
                        xc[:, sl], xx[:, sl], xlo, xhi,
                        op0=mybir.AluOpType.max, op1=mybir.AluOpType.min,
                    ), mark=f"x{h}")

        @block.sync
        def _(sync):
            sync.dma_start(w8[:], w8_d).then_inc(wdma, 16)
            cb = [0, 4, 8, 14, 16]
            for c in range(4):
                sync.dma_start(
                    pm_sb[:, cb[c]:cb[c + 1], :], pm_d[:, cb[c]:cb[c + 1], :]
                ).then_inc(pc[c], 16)
            sync.wait_ge(act, 2)
            sync.dma_start(out_d[:, 0:MH], pp[:, 0:MH]).then_inc(odma, 16)
            sync.wait_ge(odma, 32)

        @block.scalar
        def _(scalar):
            for k in range(2):
                scalar.dma_start(
                    obs_sb[:, k * 2 * F:(k + 1) * 2 * F],
                    obs_d[:, k * 2 * F:(k + 1) * 2 * F],
                ).then_inc(oc[k], 16)
            # prewarm the sigmoid table set while the stream runs
            scalar.wait_ge(dve, marks["z0"])
            nc.scalar.activation(
                scr[:], z0t[:], mybir.ActivationFunctionType.Sigmoid,
            ).then_inc(act, 1)
            for h in range(2):
                scalar.wait_ge(dve, marks[f"x{h}"])
                nc.scalar.activation(
                    pp[:, h * MH:(h + 1) * MH], xc[:, h * MH:(h + 1) * MH],
                    mybir.ActivationFunctionType.Sigmoid,
                    bias=z0t[:], scale=neg_b,
                ).then_inc(act, 1)
            scalar.wait_ge(act, 3)
            scalar.dma_start(out_d[:, MH:F], pp[:, MH:F]).then_inc(odma, 16)

        @block.tensor
        def _(tensor):
            # bias matmul first: fills PSUM [128, F] with -C/B (start=True)
            tensor.wait_ge(dve, marks["psm"])
            nc.tensor.matmul(
                psb[:, 0:F], wbias[:], onesb[:],
                start=True, stop=False, skip_group_check=True,
            ).then_inc(pe, 1)
            tensor.wait_ge(wdma, 16)
            kc = [0, 0, 1, 1, 2, 2, 2, 3]
            for k in range(8):
                tensor.wait_ge(pc[kc[k]], 16)
                nc.tensor.matmul(
                    psb[:, 0:F],
                    w8[:],
                    pm_sb[:, 2 * k:2 * (k + 1), :],
                    start=False, stop=(k == 7), skip_group_check=True,
                    perf_mode=mybir.MatmulPerfMode.DoubleRow,
                ).then_inc(pe, 1)

        block_cm.__exit__(None, None, None)

    nc.compile()
    return nc


def _scalar_constants(inputs):
    t0 = float(np.asarray(inputs["trust0"]).reshape(()))
    s0 = float(np.asarray(inputs["sigma0"]).reshape(()))
    wb = float(np.asarray(inputs["wb"]).reshape(()))
    wtp = float(np.asarray(inputs["wtp"]).reshape(()))
    st = float(np.asarray(inputs["sigma_t"]).reshape(()))
    r1 = 1.0 / math.sqrt(s0 * s0 + T * st * st)
    z0 = t0 / math.sqrt(s0 * s0)
    a_const = (t0 + T * wb + T * wtp) * r1
    b = 2.0 * wtp * r1
    c_const = a_const - z0
    b = max(b, 1e-30)           # wtp==0 guard: x carries only the C term
    vbias = -c_const / b        # PSUM pre-load so psum = s - C/B
    lo_z = math.log(0.01 / 0.99)
    hi_z = math.log(0.99 / 0.01)
    # z = z0 - B*x clamped to [lo_z, hi_z]  <=>  x in [(z0-hi_z)/B, (z0-lo_z)/B]
    xlo = (z0 - hi_z) / b
    xhi = (z0 - lo_z) / b
    # The low clip binds only if some reachable z < lo_z; the high side is
    # covered by sigmoid saturation (<= 1.02% relative vs clip at 0.99).
    z_reach_min = min(z0, z0 + c_const - 16.0 * b, z0 + c_const)
    emit_clamp = not (z_reach_min >= lo_z + 1e-6)
    return -b, z0, vbias, xlo, xhi, emit_clamp


def _shard_inputs(inputs):
    """Host-side layout + dtype casts -> per-core input maps."""
    import ml_dtypes

    obs = np.asarray(inputs["inptasksobs"])
    perf = np.asarray(inputs["inptasksperf"])
    assert obs.shape == (T, N, D) and perf.shape == (T, N, 1)

    o8 = np.zeros((NPAD, D), np.uint8)
    o8[:N] = obs[0].astype(ml_dtypes.float8_e4m3fn).view(np.uint8)
    ow = o8.view(np.uint32)                       # [NPAD, 4]

    p8 = np.zeros((T, NPAD), np.uint8)
    p8[:, :N] = perf[:, :, 0].astype(ml_dtypes.float8_e4m3fn).view(np.uint8)

    # DoubleRow selection weights: w8[p, j, m] = (m == j*64 + p%64)
    w = np.zeros((128, 2, 128), np.uint8)
    one = np.uint8(0x38)                          # fp8-e4m3 1.0
    for p in range(128):
        for j in range(2):
            w[p, j, j * 64 + (p % 64)] = one

    in_maps = []
    for c in range(NCORES):
        oc = np.ascontiguousarray(
            ow[c * PER:(c + 1) * PER].reshape(128, F, 4).reshape(128, 4 * F)
        )
        # matmul a sums t in {2a, 2a+1} over all 128 psum rows:
        # pm[tl*64+v, 2a+j, n] = perf[2a+tl, (j*64 + v)*490 + n]
        xc = p8[:, c * PER:(c + 1) * PER].reshape(8, 2, 2, 64, F)  # [a,tl,j,v,n]
        pm = np.ascontiguousarray(
            xc.transpose(1, 3, 0, 2, 4).reshape(128, T * F)
        )
        in_maps.append({
            "obsw": oc,
            "perfc": pm.view(ml_dtypes.float8_e4m3fn).reshape(128, T, F),
            "wsel": w.view(ml_dtypes.float8_e4m3fn),
        })
    return in_maps


def run(inputs, trace=False, **kw):
    """Shard, run on 8 cores, gather. Returns (output [N,1] f32, exec_time_ns)."""
    from concourse.bass_utils import run_bass_kernel_spmd

    neg_b, z0, vbias, xlo, xhi, emit_clamp = _scalar_constants(inputs)
    nc = build_program(neg_b, z0, vbias, xlo, xhi, emit_clamp)
    in_maps = _shard_inputs(inputs)

    res = run_bass_kernel_spmd(
        nc, in_maps, core_ids=list(range(NCORES)), trace=trace, **kw
    )
    full = np.concatenate(
        [res.results[c]["out"].astype(np.float32).reshape(-1)
         for c in range(NCORES)]
    )
    return full[:N].reshape(N, 1).astype(np.float32, copy=False), res.exec_time_ns


def kernel(**inputs):
    out, _ = run(inputs, trace=False)
    return out
